# revision 7
# baseline (speedup 1.0000x reference)
"""Trainium2 Bass kernel for nn_DNCClassifier_82635170775168.

Key observation: in the reference DNC, the controller input is
``cat(x_t, zeros)`` every step (the ixaxaar dnc.py bug: read vectors are
never fed back), so the LSTM state (h, c) evolves independently of the
DNC memory subsystem, and the output ``h_T @ W_fc.T + b_fc`` depends only
on the LSTM path.  The external-memory machinery (usage, allocation,
temporal links, read weights) is dead code w.r.t. the output, so this
kernel computes just the LSTM recurrence + final linear layer.

Sharding: pure data parallel, batch 128 -> 16 per core across 8 cores.

Per-core design (feature-major: gate rows on partitions, batch on free):
  - per step t, psum[f, b] (128 x 128: 8 gate-row chunks x 16 batch) is
    built as  I @ U[t]  (identity matmul, start=True — lays down the
    precomputed x-projection + bias, and can run during the previous
    step's tail since it doesn't depend on h)  followed by 16 accumulating
    weight matmuls  W_hh.T[k-tile, chunk] @ h[k-tile].
  - gate rows are permuted to chunk order [i, f, o, g]; the g-gate rows of
    W_hh / W_x / bias are pre-scaled by 2 on the host so a single Sigmoid
    activation over all 128 psum columns yields sigma for i,f,o and
    sigma(2g) for g, with tanh(g) = 2*sigma(2g) - 1 recovered on the DVE.
  - U[t] = W_x.T @ [x_t; 1] is precomputed for all t by 128 matmuls
    contracting over K=28, stored feature-major in SBUF (two phases).
"""

import sys

if "/opt/trn_rl_repo" not in sys.path:
    sys.path.insert(0, "/opt/trn_rl_repo")

import numpy as np

B_FULL = 128
N_CORES = 8
B = B_FULL // N_CORES   # 16 batch per core
T = 512
H = 256
G = 4 * H               # 1024 gate rows
IN = 27
INX = IN + 1            # + ones row for bias
OUT = 128
NCHUNK = 8              # gate-row chunks of 128
TB = 32                 # precompute time-block (32 steps x 16 batch = 512 cols)

W_DTYPE = "bfloat16"    # dtype of W_hh tiles and h (recurrent matmul)
U_DTYPE = "float32"     # dtype of U and the identity matmul


def _mybir_dt(name):
    import concourse.mybir as mybir

    return getattr(mybir.dt, name)


def build(t_steps=T, w_dtype=W_DTYPE, u_dtype=U_DTYPE, repeat=1):
    """Builds the per-core Bass program. Returns the Bacc instance.

    repeat > 1 re-runs the recurrence loop (timing-only builds: the extra
    passes reuse U and carry the h/c state on, so outputs are meaningless
    but per-pass timing is identical)."""
    import concourse.mybir as mybir
    from concourse import bacc
    from concourse.tile import TileContext

    assert t_steps % (2 * TB) == 0
    tph = t_steps // 2          # steps per phase
    nblk = tph // TB            # time blocks per phase

    fp32 = mybir.dt.float32
    wdt = _mybir_dt(w_dtype)
    udt = _mybir_dt(u_dtype)
    AFT = mybir.ActivationFunctionType
    ALU = mybir.AluOpType

    nc = bacc.Bacc("TRN2")

    d_xT = nc.dram_tensor("xT", [INX, t_steps * B], fp32, kind="ExternalInput")
    d_whh = nc.dram_tensor("whh", [128, 16 * 128], wdt, kind="ExternalInput")
    d_wx = nc.dram_tensor("wx", [INX, G], fp32, kind="ExternalInput")
    d_ident = nc.dram_tensor("ident", [128, 128], udt, kind="ExternalInput")
    d_wfc = nc.dram_tensor("wfc", [128, 2 * 128], fp32, kind="ExternalInput")
    d_bfc = nc.dram_tensor("bfc", [128, 1], fp32, kind="ExternalInput")
    d_y = nc.dram_tensor("y", [OUT, B], fp32, kind="ExternalOutput")

    with TileContext(nc) as tc:
        with (
            tc.tile_pool(name="persist", bufs=1) as persist,
            tc.tile_pool(name="state", bufs=2) as state,
            tc.tile_pool(name="work", bufs=3) as work,
            tc.tile_pool(name="pp_pre", bufs=2, space="PSUM") as pp_pre,
            tc.tile_pool(name="pp_gates", bufs=2, space="PSUM") as pp_gates,
            tc.tile_pool(name="pp_fc", bufs=1, space="PSUM") as pp_fc,
        ):
            s_xT = persist.tile([INX, t_steps * B], fp32)
            s_whh = persist.tile([128, 16 * 128], wdt)
            s_wx = persist.tile([INX, G], fp32)
            s_ident = persist.tile([128, 128], udt)
            s_wfc = persist.tile([128, 2 * 128], fp32)
            s_bfc = persist.tile([128, 1], fp32)
            s_U = persist.tile([128, tph * 128], udt)

            nc.sync.dma_start(out=s_xT[:], in_=d_xT[:])
            nc.sync.dma_start(out=s_whh[:], in_=d_whh[:])
            nc.sync.dma_start(out=s_wx[:], in_=d_wx[:])
            nc.sync.dma_start(out=s_ident[:], in_=d_ident[:])
            nc.sync.dma_start(out=s_wfc[:], in_=d_wfc[:])
            nc.sync.dma_start(out=s_bfc[:], in_=d_bfc[:])

            U4 = s_U[:].rearrange("p (t c b) -> p t c b", c=NCHUNK, b=B)

            h_cur = state.tile([128, 32], wdt, tag="h")
            c_cur = state.tile([128, 32], fp32, tag="c")
            nc.vector.memset(h_cur[:], 0.0)
            nc.vector.memset(c_cur[:], 0.0)

            def precompute(phase):
                for tb in range(nblk):
                    t0 = phase * tph + tb * TB
                    rhs = s_xT[:, t0 * B : (t0 + TB) * B]
                    for c in range(NCHUNK):
                        ps = pp_pre.tile([128, TB * B], fp32, tag="ps_pre")
                        nc.tensor.matmul(
                            ps[:],
                            s_wx[:, c * 128 : (c + 1) * 128],
                            rhs,
                            start=True,
                            stop=True,
                        )
                        psv = ps[:].rearrange("p (t b) -> p t b", b=B)
                        dst = U4[:, tb * TB : (tb + 1) * TB, c, :]
                        if c % 2 == 0:
                            nc.vector.tensor_copy(out=dst, in_=psv)
                        else:
                            nc.scalar.copy(out=dst, in_=psv)

            def step(tl):
                nonlocal h_cur, c_cur
                ps = pp_gates.tile([128, 128], fp32, tag="ps_gates")
                # identity matmul lays down U[t] (+bias); no h dependency
                nc.tensor.matmul(
                    ps[:],
                    s_ident[:],
                    s_U[:, tl * 128 : (tl + 1) * 128],
                    start=True,
                    stop=False,
                )
                for c in range(NCHUNK):
                    for kt in range(2):
                        nc.tensor.matmul(
                            ps[:, c * B : (c + 1) * B],
                            s_whh[:, (kt * 8 + c) * 128 : (kt * 8 + c + 1) * 128],
                            h_cur[:, kt * B : (kt + 1) * B],
                            start=False,
                            stop=(c == NCHUNK - 1 and kt == 1),
                            skip_group_check=True,
                        )
                # one Sigmoid over all gates: cols [i|f|o|sigma(2g)]
                sig = work.tile([128, 128], fp32, tag="sig")
                nc.scalar.activation(sig[:], ps[:], AFT.Sigmoid)

                # tanh(g) = 2*sigma(2g) - 1
                q = work.tile([128, 32], fp32, tag="q")
                nc.vector.tensor_scalar(
                    out=q[:], in0=sig[:, 96:128], scalar1=2.0, scalar2=1.0,
                    op0=ALU.mult, op1=ALU.subtract,
                )
                c_new = state.tile([128, 32], fp32, tag="c")
                nc.vector.tensor_mul(out=c_new[:], in0=c_cur[:], in1=sig[:, 32:64])
                t1 = work.tile([128, 32], fp32, tag="t1")
                nc.vector.tensor_mul(out=t1[:], in0=sig[:, 0:32], in1=q[:])
                nc.vector.tensor_add(out=c_new[:], in0=c_new[:], in1=t1[:])
                th = work.tile([128, 32], fp32, tag="th")
                nc.scalar.activation(th[:], c_new[:], AFT.Tanh)
                h_new = state.tile([128, 32], wdt, tag="h")
                nc.vector.tensor_mul(out=h_new[:], in0=sig[:, 64:96], in1=th[:])
                h_cur, c_cur = h_new, c_new

            for phase in range(2):
                precompute(phase)
                for tl in range(tph):
                    step(tl)
            for _rep in range(repeat - 1):
                for phase in range(2):
                    for tl in range(tph):
                        step(tl)

            # ---- classifier head: logits[o, b] = W_fc @ h + b_fc
            ps_fc = pp_fc.tile([128, B], fp32)
            h_fc = h_cur
            if w_dtype != "float32":
                h_fc = work.tile([128, 32], fp32, tag="h_fc32")
                nc.vector.tensor_copy(out=h_fc[:], in_=h_cur[:])
            for kt in range(2):
                nc.tensor.matmul(
                    ps_fc[:],
                    s_wfc[:, kt * 128 : (kt + 1) * 128],
                    h_fc[:, kt * B : (kt + 1) * B],
                    start=(kt == 0),
                    stop=(kt == 1),
                )
            out_sb = work.tile([128, B], fp32, tag="out_sb")
            nc.scalar.activation(
                out_sb[:], ps_fc[:], AFT.Identity, bias=s_bfc[:, 0:1]
            )
            nc.sync.dma_start(out=d_y[:], in_=out_sb[:])

    nc.compile()
    return nc


def prep_core_inputs(x, W_ih, W_hh, b_ih, b_hh, W_fc, b_fc, t_steps=T,
                     w_dtype=W_DTYPE, u_dtype=U_DTYPE):
    """Host-side layout prep. Returns list of per-core input dicts."""
    import ml_dtypes

    def npdt(name):
        return np.float32 if name == "float32" else ml_dtypes.bfloat16

    x = np.ascontiguousarray(np.asarray(x, dtype=np.float32))
    W_ih = np.asarray(W_ih, dtype=np.float32)
    W_hh = np.asarray(W_hh, dtype=np.float32)
    bias = np.asarray(b_ih, dtype=np.float32) + np.asarray(b_hh, dtype=np.float32)
    W_fc = np.asarray(W_fc, dtype=np.float32)
    b_fc = np.asarray(b_fc, dtype=np.float32)

    # gate-row permutation: torch order [i, f, g, o] -> chunk order [i, f, o, g]
    perm = np.r_[0 : 2 * H, 3 * H : 4 * H, 2 * H : 3 * H]
    Wp_hh = W_hh[perm].copy()         # (1024, 256)
    Wp_ihx = W_ih[perm, :IN].copy()   # (1024, 27)
    bias_p = bias[perm].copy()        # (1024,)
    # pre-scale g rows (chunks 6,7 = permuted rows 768:1024) by 2 so that
    # sigmoid(2g) is computed and tanh(g) = 2*sigmoid(2g) - 1
    Wp_hh[768:] *= 2.0
    Wp_ihx[768:] *= 2.0
    bias_p[768:] *= 2.0

    whh_host = np.empty((128, 16 * 128), dtype=np.float32)
    for kt in range(2):
        for c in range(NCHUNK):
            blk = Wp_hh[c * 128 : (c + 1) * 128, kt * 128 : (kt + 1) * 128].T
            whh_host[:, (kt * 8 + c) * 128 : (kt * 8 + c + 1) * 128] = blk
    whh_host = whh_host.astype(npdt(w_dtype))

    wx_host = np.empty((INX, G), dtype=np.float32)
    wx_host[:IN] = Wp_ihx.T
    wx_host[IN] = bias_p

    ident_host = np.eye(128, dtype=np.float32).astype(npdt(u_dtype))

    wfc_host = np.empty((128, 2 * 128), dtype=np.float32)
    for kt in range(2):
        wfc_host[:, kt * 128 : (kt + 1) * 128] = W_fc[:, kt * 128 : (kt + 1) * 128].T
    bfc_host = b_fc.reshape(128, 1)

    in_maps = []
    for core in range(N_CORES):
        xc = x[core * B : (core + 1) * B, :t_steps, :]        # (16, t, 27)
        xT = np.empty((INX, t_steps * B), dtype=np.float32)
        xT[:IN] = xc.transpose(2, 1, 0).reshape(IN, t_steps * B)
        xT[IN] = 1.0
        in_maps.append(
            dict(
                xT=np.ascontiguousarray(xT),
                whh=whh_host,
                wx=wx_host,
                ident=ident_host,
                wfc=wfc_host,
                bfc=bfc_host,
            )
        )
    return in_maps


_NC_CACHE = {}


def _get_nc(t_steps=T, w_dtype=W_DTYPE, u_dtype=U_DTYPE, repeat=1):
    key = (t_steps, w_dtype, u_dtype, repeat)
    if key not in _NC_CACHE:
        _NC_CACHE[key] = build(t_steps, w_dtype, u_dtype, repeat)
    return _NC_CACHE[key]


def kernel(**inputs):
    from concourse.bass_utils import run_bass_kernel_spmd

    nc = _get_nc()
    in_maps = prep_core_inputs(
        inputs["x"],
        inputs["W_ih"],
        inputs["W_hh"],
        inputs["b_ih"],
        inputs["b_hh"],
        inputs["W_fc"],
        inputs["b_fc"],
    )
    res = run_bass_kernel_spmd(nc, in_maps, core_ids=list(range(N_CORES)))
    out = np.empty((B_FULL, OUT), dtype=np.float32)
    for core in range(N_CORES):
        out[core * B : (core + 1) * B, :] = res.results[core]["y"].T
    return out


# revision 31
# speedup vs baseline: 6.1522x; 6.1522x over previous
"""Trainium2 Bass kernel for nn_DNCClassifier_82635170775168.

Key observation: in the reference DNC, the controller input is
``cat(x_t, zeros)`` every step (the ixaxaar dnc.py bug: read vectors are
never fed back), so the LSTM state (h, c) evolves independently of the
DNC memory subsystem, and the output ``h_T @ W_fc.T + b_fc`` depends only
on the LSTM path.  The external-memory machinery (usage, allocation,
temporal links, read weights) is dead code w.r.t. the output, so this
kernel computes just the LSTM recurrence + final linear layer.

Sharding: pure data parallel, batch 128 -> 16 per core across 8 cores.

Per-core design (feature-major: gate rows on partitions, batch on free):
  - per step t, psum[f, b] (128 x 128: 8 gate-row chunks x 16 batch) is
    built as  I @ U[t]  (identity matmul, start=True — lays down the
    precomputed x-projection + bias, and can run during the previous
    step's tail since it doesn't depend on h)  followed by 16 accumulating
    weight matmuls  W_hh.T[k-tile, chunk] @ h[k-tile].
  - gate rows are permuted to chunk order [i, f, o, g]; the g-gate rows of
    W_hh / W_x / bias are pre-scaled by 2 on the host so a single Sigmoid
    activation over all 128 psum columns yields sigma for i,f,o and
    sigma(2g) for g, with tanh(g) = 2*sigma(2g) - 1 recovered on the DVE.
  - U[t] = W_x.T @ [x_t; 1] is precomputed for all t by 128 matmuls
    contracting over K=28, stored feature-major in SBUF (two phases).
"""

import sys

if "/opt/trn_rl_repo" not in sys.path:
    sys.path.insert(0, "/opt/trn_rl_repo")

import numpy as np

B_FULL = 128
N_CORES = 8
B = B_FULL // N_CORES   # 16 batch per core
T = 512
H = 256
G = 4 * H               # 1024 gate rows
IN = 27
INX = IN + 1            # + ones row for bias
OUT = 128
NCHUNK = 8              # gate-row chunks of 128
TB = 32                 # precompute time-block (32 steps x 16 batch = 512 cols)

W_DTYPE = "bfloat16"    # dtype of W_hh tiles and h (recurrent matmul)
U_DTYPE = "float32"     # dtype of U and the identity matmul
# float32r streams the moving operand at full rate for N>=256 while keeping
# fp32 precision on hardware (verified: same rel err as float32 here)
X_DTYPE = "float32r"    # dtype of the xT/W_x operands of the precompute matmuls


def _mybir_dt(name):
    import concourse.mybir as mybir

    return getattr(mybir.dt, name)


def build(t_steps=T, w_dtype=W_DTYPE, u_dtype=U_DTYPE, repeat=1,
          x_dtype=X_DTYPE):
    """Builds the per-core Bass program. Returns the Bacc instance.

    repeat > 1 re-runs the recurrence loop (timing-only builds: the extra
    passes reuse U and carry the h/c state on, so outputs are meaningless
    but per-pass timing is identical)."""
    import concourse.mybir as mybir
    from concourse import bacc
    from concourse.tile import TileContext

    assert t_steps % (2 * TB) == 0
    tph = t_steps // 2          # steps per phase
    nblk = tph // TB            # time blocks per phase

    fp32 = mybir.dt.float32
    wdt = _mybir_dt(w_dtype)
    udt = _mybir_dt(u_dtype)
    xdt = _mybir_dt(x_dtype)
    AFT = mybir.ActivationFunctionType
    ALU = mybir.AluOpType

    nc = bacc.Bacc("TRN2")

    d_xT = nc.dram_tensor("xT", [INX, t_steps * B], xdt, kind="ExternalInput")
    d_whh = nc.dram_tensor("whh", [128, 16 * 128], wdt, kind="ExternalInput")
    d_wx = nc.dram_tensor("wx", [INX, G], xdt, kind="ExternalInput")
    d_ident = nc.dram_tensor("ident", [128, 128], udt, kind="ExternalInput")
    d_wfc = nc.dram_tensor("wfc", [128, 2 * 128], fp32, kind="ExternalInput")
    d_bfc = nc.dram_tensor("bfc", [128, 1], fp32, kind="ExternalInput")
    d_y = nc.dram_tensor("y", [OUT, B], fp32, kind="ExternalOutput")

    with TileContext(nc) as tc:
        with (
            tc.tile_pool(name="persist", bufs=1) as persist,
            tc.tile_pool(name="state", bufs=2) as state,
            tc.tile_pool(name="work", bufs=3) as work,
            tc.tile_pool(name="pp_pre", bufs=2, space="PSUM") as pp_pre,
            tc.tile_pool(name="pp_g", bufs=2, space="PSUM") as pp_g,
            tc.tile_pool(name="pp_ifo", bufs=2, space="PSUM") as pp_ifo,
            tc.tile_pool(name="pp_fc", bufs=1, space="PSUM") as pp_fc,
        ):
            s_xT = persist.tile([INX, t_steps * B], xdt)
            s_whh = persist.tile([128, 16 * 128], wdt)
            s_wx = persist.tile([INX, G], xdt)
            s_ident = persist.tile([128, 128], udt)
            s_wfc = persist.tile([128, 2 * 128], fp32)
            s_bfc = persist.tile([128, 1], fp32)
            # one U tile per 32-step block so phase-1 blocks can be
            # recomputed into them as soon as phase-0 readers finish
            u_tiles = [
                persist.tile([128, TB * 128], udt, tag=f"U{tb}", name=f"U{tb}")
                for tb in range(nblk)
            ]

            nc.sync.dma_start(out=s_xT[:], in_=d_xT[:])
            nc.sync.dma_start(out=s_whh[:], in_=d_whh[:])
            nc.sync.dma_start(out=s_wx[:], in_=d_wx[:])
            nc.sync.dma_start(out=s_ident[:], in_=d_ident[:])
            nc.sync.dma_start(out=s_wfc[:], in_=d_wfc[:])
            nc.sync.dma_start(out=s_bfc[:], in_=d_bfc[:])

            h_cur = state.tile([128, 32], wdt, tag="h")
            # A holds [tanh(g) | c] so one wide DVE mul against the adjacent
            # [sigma_i | sigma_f] columns of sig yields both cell products
            A_cur = state.tile([128, 64], fp32, tag="A")
            nc.vector.memset(h_cur[:], 0.0)
            nc.vector.memset(A_cur[:], 0.0)

            def precompute_block(phase, tb):
                # U[t] for the 32 steps of block (phase, tb) into u_tiles[tb]
                t0 = phase * tph + tb * TB
                rhs = s_xT[:, t0 * B : (t0 + TB) * B]
                U4 = u_tiles[tb][:].rearrange(
                    "p (t c b) -> p t c b", c=NCHUNK, b=B
                )
                for c in range(NCHUNK):
                    ps = pp_pre.tile([128, TB * B], fp32, tag="ps_pre")
                    nc.tensor.matmul(
                        ps[:],
                        s_wx[:, c * 128 : (c + 1) * 128],
                        rhs,
                        start=True,
                        stop=True,
                    )
                    psv = ps[:].rearrange("p (t b) -> p t b", b=B)
                    # split the psum evacuation so an interleaved copy can
                    # only delay the step chain by ~half a copy
                    for half in range(2):
                        dst = U4[:, tb_half(half), c, :]
                        src = psv[:, tb_half(half), :]
                        if (c + half) % 2 == 0:
                            nc.vector.tensor_copy(out=dst, in_=src)
                        else:
                            nc.scalar.copy(out=dst, in_=src)

            def tb_half(half):
                return slice(half * (TB // 2), (half + 1) * (TB // 2))

            def step(tl):
                nonlocal h_cur, A_cur
                # g chunks get their own psum bank so tanh(g) runs on ACT
                # while the i,f,o matmuls are still streaming
                ps_g = pp_g.tile([128, 32], fp32, tag="ps_g")
                ps_ifo = pp_ifo.tile([128, 96], fp32, tag="ps_ifo")
                # identity matmuls lay down U[t] (+bias); no h dependency
                ublk = u_tiles[tl // TB]
                off = (tl % TB) * 128
                nc.tensor.matmul(
                    ps_g[:], s_ident[:],
                    ublk[:, off + 96 : off + 128],
                    start=True, stop=False,
                )
                for ci, c in enumerate((6, 7)):
                    for kt in range(2):
                        nc.tensor.matmul(
                            ps_g[:, ci * B : (ci + 1) * B],
                            s_whh[:, (kt * 8 + c) * 128 : (kt * 8 + c + 1) * 128],
                            h_cur[:, kt * B : (kt + 1) * B],
                            start=False,
                            stop=(ci == 1 and kt == 1),
                            skip_group_check=True,
                        )
                nc.tensor.matmul(
                    ps_ifo[:], s_ident[:],
                    ublk[:, off : off + 96],
                    start=True, stop=False,
                )
                for c in range(6):
                    for kt in range(2):
                        nc.tensor.matmul(
                            ps_ifo[:, c * B : (c + 1) * B],
                            s_whh[:, (kt * 8 + c) * 128 : (kt * 8 + c + 1) * 128],
                            h_cur[:, kt * B : (kt + 1) * B],
                            start=False,
                            stop=(c == 5 and kt == 1),
                            skip_group_check=True,
                        )
                # tanh(g) straight off psum into A[:, 0:32] (overlaps ifo MMs)
                nc.scalar.activation(A_cur[:, 0:32], ps_g[:], AFT.Tanh)
                sig = work.tile([128, 96], fp32, tag="sig")
                nc.scalar.activation(sig[:], ps_ifo[:], AFT.Sigmoid)

                # prod = [tanh_g | c] * [sigma_i | sigma_f] in one wide op
                prod = work.tile([128, 64], fp32, tag="prod")
                nc.vector.tensor_mul(out=prod[:], in0=A_cur[:], in1=sig[:, 0:64])
                A_new = state.tile([128, 64], fp32, tag="A")
                nc.vector.tensor_add(
                    out=A_new[:, 32:64], in0=prod[:, 0:32], in1=prod[:, 32:64]
                )
                th = work.tile([128, 32], fp32, tag="th")
                nc.scalar.activation(th[:], A_new[:, 32:64], AFT.Tanh)
                h_new = state.tile([128, 32], wdt, tag="h")
                nc.vector.tensor_mul(out=h_new[:], in0=sig[:, 64:96], in1=th[:])
                h_cur, A_cur = h_new, A_new

            # block (0,0) first; the rest trickle into step-loop idle slots.
            # Phase-1 blocks reuse u_tiles[tb]: emitted only after every
            # phase-0 step that reads the tile, so their WAR dependency is
            # already satisfied and they never head-of-line-block the chain.
            precompute_block(0, 0)
            pending = [(0, tb) for tb in range(1, nblk)] + [
                (1, tb) for tb in range(nblk)
            ]
            for g in range(t_steps):
                phase, tl = divmod(g, tph)
                if g % 4 == 2 and pending:
                    for i, blk in enumerate(pending):
                        ph_b, tb_b = blk
                        if ph_b == 0 or g >= (tb_b + 1) * TB + 1:
                            precompute_block(ph_b, tb_b)
                            pending.pop(i)
                            break
                step(tl)
            assert not pending, pending
            for _rep in range(repeat - 1):
                for g in range(t_steps):
                    step(g % tph)

            # ---- classifier head: logits[o, b] = W_fc @ h + b_fc
            ps_fc = pp_fc.tile([128, B], fp32)
            h_fc = h_cur
            if w_dtype != "float32":
                h_fc = work.tile([128, 32], fp32, tag="h_fc32")
                nc.vector.tensor_copy(out=h_fc[:], in_=h_cur[:])
            for kt in range(2):
                nc.tensor.matmul(
                    ps_fc[:],
                    s_wfc[:, kt * 128 : (kt + 1) * 128],
                    h_fc[:, kt * B : (kt + 1) * B],
                    start=(kt == 0),
                    stop=(kt == 1),
                )
            out_sb = work.tile([128, B], fp32, tag="out_sb")
            nc.scalar.activation(
                out_sb[:], ps_fc[:], AFT.Identity, bias=s_bfc[:, 0:1]
            )
            nc.sync.dma_start(out=d_y[:], in_=out_sb[:])

    nc.compile()
    return nc


def prep_core_inputs(x, W_ih, W_hh, b_ih, b_hh, W_fc, b_fc, t_steps=T,
                     w_dtype=W_DTYPE, u_dtype=U_DTYPE, x_dtype=X_DTYPE):
    """Host-side layout prep. Returns list of per-core input dicts."""
    import ml_dtypes

    def npdt(name):
        return ml_dtypes.bfloat16 if name == "bfloat16" else np.float32

    x = np.ascontiguousarray(np.asarray(x, dtype=np.float32))
    W_ih = np.asarray(W_ih, dtype=np.float32)
    W_hh = np.asarray(W_hh, dtype=np.float32)
    bias = np.asarray(b_ih, dtype=np.float32) + np.asarray(b_hh, dtype=np.float32)
    W_fc = np.asarray(W_fc, dtype=np.float32)
    b_fc = np.asarray(b_fc, dtype=np.float32)

    # gate-row permutation: torch order [i, f, g, o] -> chunk order [i, f, o, g]
    perm = np.r_[0 : 2 * H, 3 * H : 4 * H, 2 * H : 3 * H]
    Wp_hh = W_hh[perm].copy()         # (1024, 256)
    Wp_ihx = W_ih[perm, :IN].copy()   # (1024, 27)
    bias_p = bias[perm].copy()        # (1024,)

    whh_host = np.empty((128, 16 * 128), dtype=np.float32)
    for kt in range(2):
        for c in range(NCHUNK):
            blk = Wp_hh[c * 128 : (c + 1) * 128, kt * 128 : (kt + 1) * 128].T
            whh_host[:, (kt * 8 + c) * 128 : (kt * 8 + c + 1) * 128] = blk
    whh_host = whh_host.astype(npdt(w_dtype))

    wx_host = np.empty((INX, G), dtype=np.float32)
    wx_host[:IN] = Wp_ihx.T
    wx_host[IN] = bias_p
    wx_host = wx_host.astype(npdt(x_dtype))

    ident_host = np.eye(128, dtype=np.float32).astype(npdt(u_dtype))

    wfc_host = np.empty((128, 2 * 128), dtype=np.float32)
    for kt in range(2):
        wfc_host[:, kt * 128 : (kt + 1) * 128] = W_fc[:, kt * 128 : (kt + 1) * 128].T
    bfc_host = b_fc.reshape(128, 1)

    in_maps = []
    for core in range(N_CORES):
        xc = x[core * B : (core + 1) * B, :t_steps, :]        # (16, t, 27)
        xT = np.empty((INX, t_steps * B), dtype=np.float32)
        xT[:IN] = xc.transpose(2, 1, 0).reshape(IN, t_steps * B)
        xT[IN] = 1.0
        in_maps.append(
            dict(
                xT=np.ascontiguousarray(xT.astype(npdt(x_dtype))),
                whh=whh_host,
                wx=wx_host,
                ident=ident_host,
                wfc=wfc_host,
                bfc=bfc_host,
            )
        )
    return in_maps


_NC_CACHE = {}


def _get_nc(t_steps=T, w_dtype=W_DTYPE, u_dtype=U_DTYPE, repeat=1):
    key = (t_steps, w_dtype, u_dtype, repeat)
    if key not in _NC_CACHE:
        _NC_CACHE[key] = build(t_steps, w_dtype, u_dtype, repeat)
    return _NC_CACHE[key]


def kernel(**inputs):
    from concourse.bass_utils import run_bass_kernel_spmd

    nc = _get_nc()
    in_maps = prep_core_inputs(
        inputs["x"],
        inputs["W_ih"],
        inputs["W_hh"],
        inputs["b_ih"],
        inputs["b_hh"],
        inputs["W_fc"],
        inputs["b_fc"],
    )
    res = run_bass_kernel_spmd(nc, in_maps, core_ids=list(range(N_CORES)))
    out = np.empty((B_FULL, OUT), dtype=np.float32)
    for core in range(N_CORES):
        out[core * B : (core + 1) * B, :] = res.results[core]["y"].T
    return out
